# revision 6
# baseline (speedup 1.0000x reference)
"""Causal multi-head attention block (B=4,S=2048,D=1024,H=16) on 8 trn2 cores.

Sharding: data-parallel over batch (4) x tensor-parallel over head-groups (2).
Core c handles batch c//2, heads [8*(c%2), 8*(c%2)+8). Each core computes its
partial output projection; the host sums the two head-group partials per batch.
"""

import sys

for p in ("/opt/trn_rl_repo", "/root/.axon_site/_ro/trn_rl_repo"):
    if p not in sys.path:
        sys.path.insert(0, p)

import numpy as np
import ml_dtypes

import concourse.bass as bass
import concourse.mybir as mybir
import concourse.tile as tile
from concourse import bacc
from concourse.bass_utils import run_bass_kernel_spmd

FP32 = mybir.dt.float32
BF16 = mybir.dt.bfloat16
AF = mybir.ActivationFunctionType

B, S, D, H = 4, 2048, 1024, 16
DH = 64
N_CORES = 8
HPC = H // 2  # heads per core (head-group tensor parallel = 2)


def build_nc(s=S, d=D, hpc=HPC, dbg=False):
    """Build the per-core SPMD program. All 8 cores run this same program."""
    P = 128
    KC = d // P              # feature chunks (contraction for qkv proj)
    NPAIR = hpc // 2         # head pairs
    VC = hpc * DH            # v columns / a columns per core
    QTS = 512                # query tile size
    NQT = s // QTS           # query tiles
    PC = VC // P             # proj contraction chunks (= NPAIR)
    NOUT = d // 512          # out-proj n tiles

    nc = bacc.Bacc("TRN2", target_bir_lowering=False, debug=False,
                   num_devices=N_CORES)

    xb = nc.dram_tensor("xb", [s, d], FP32, kind="ExternalInput")
    wqk = nc.dram_tensor("wqk", [d, 2 * VC], FP32, kind="ExternalInput")
    wv = nc.dram_tensor("wv", [d, VC], FP32, kind="ExternalInput")
    wp = nc.dram_tensor("wp", [VC, d], FP32, kind="ExternalInput")
    bqk = nc.dram_tensor("bqk", [2 * VC], FP32, kind="ExternalInput")
    bv = nc.dram_tensor("bv", [VC], FP32, kind="ExternalInput")
    bph = nc.dram_tensor("bph", [d], FP32, kind="ExternalInput")
    yp = nc.dram_tensor("yp", [s, d], FP32, kind="ExternalOutput")

    xbf = nc.dram_tensor("xbf", [s, d], BF16)  # bf16 copy of x (internal)
    rsd = nc.dram_tensor("rsd", [hpc, 512], FP32)  # recip-sum staging

    if dbg:
        qTo = nc.dram_tensor("qTo", [128, hpc // 2, s], FP32, kind="ExternalOutput")
        kTo = nc.dram_tensor("kTo", [128, hpc // 2, s], FP32, kind="ExternalOutput")
        vo = nc.dram_tensor("vo", [128, s // 128, hpc, DH + 1], FP32, kind="ExternalOutput")
        aTno = nc.dram_tensor("aTno", [128, hpc // 2, s], FP32, kind="ExternalOutput")

    band_np = np.triu(np.ones((P, P), dtype=ml_dtypes.bfloat16))
    band_dram = nc.inline_tensor(band_np, name="band")

    with tile.TileContext(nc) as tc:
        with (
            tc.tile_pool(name="singles", bufs=1) as singles,
            tc.tile_pool(name="xt", bufs=12) as xt_pool,
            tc.tile_pool(name="probs", bufs=2) as probs_pool,
            tc.tile_pool(name="norm", bufs=4) as norm_pool,
            tc.tile_pool(name="ysb", bufs=2) as y_pool,
            tc.tile_pool(name="mm512", bufs=2, space="PSUM") as mm_ps,
            tc.tile_pool(name="scps", bufs=1, space="PSUM") as sc_ps,
            tc.tile_pool(name="atps", bufs=1, space="PSUM") as at_ps,
        ):
            # ---- persistent SBUF state ----
            wqk_sb = singles.tile([P, KC, 2 * VC], BF16)
            wv_sb = singles.tile([P, KC, VC], BF16)
            wp_sb = singles.tile([P, PC, d], BF16)
            bqk_sb = singles.tile([P, 2 * VC // P], FP32)
            bv_rep = singles.tile([P, VC], FP32)
            bp_rep = singles.tile([P, d], FP32)
            band_sb = singles.tile([P, P], BF16)
            qT = singles.tile([P, NPAIR, s], BF16)
            kT = singles.tile([P, NPAIR, s], BF16)
            v_sb = singles.tile([P, s // P, hpc, DH + 1], BF16)
            aTn = singles.tile([P, NPAIR, s], BF16)

            # ---- constant / weight loads ----
            nc.sync.dma_start(band_sb[:], band_dram[:])
            for kc in range(KC):
                nc.gpsimd.dma_start(out=wqk_sb[:, kc, :],
                                    in_=wqk[kc * P:(kc + 1) * P, :])
                nc.gpsimd.dma_start(out=wv_sb[:, kc, :],
                                    in_=wv[kc * P:(kc + 1) * P, :])
            for pc in range(PC):
                nc.gpsimd.dma_start(out=wp_sb[:, pc, :],
                                    in_=wp[pc * P:(pc + 1) * P, :])
            nc.sync.dma_start(out=bqk_sb[:],
                              in_=bqk.rearrange("(ct p) -> p ct", p=P))
            nc.sync.dma_start(out=bv_rep[:], in_=bv.rearrange("(a b) -> a b", a=1).to_broadcast((P, VC)))
            nc.sync.dma_start(out=bp_rep[:], in_=bph.rearrange("(a b) -> a b", a=1).to_broadcast((P, d)))
            # ones column of v' (fused row-sum trick)
            nc.vector.memset(v_sb[:, :, :, DH], 1.0)

            # x -> bf16 (cast DMA), chunked per query tile for pipelining
            for tt in range(NQT):
                nc.gpsimd.dma_start(out=xbf[tt * QTS:(tt + 1) * QTS, :],
                                    in_=xb[tt * QTS:(tt + 1) * QTS, :])

            for tt in range(NQT):
                ts0 = tt * QTS
                # ---- load x^T tiles (feature-major) for this token tile ----
                xts = []
                for kc in range(KC):
                    xt = xt_pool.tile([P, QTS], BF16)
                    nc.sync.dma_start(
                        out=xt[:],
                        in_=xbf[ts0:ts0 + QTS, kc * P:(kc + 1) * P],
                        transpose=True)
                    xts.append(xt)

                # ---- q^T / k^T projection (feature-major out) ----
                for ct in range(2 * VC // P):
                    ps = mm_ps.tile([P, QTS], FP32, tag="mm512")
                    for kc in range(KC):
                        nc.tensor.matmul(
                            ps[:], wqk_sb[:, kc, ct * P:(ct + 1) * P],
                            xts[kc][:], start=(kc == 0), stop=(kc == KC - 1))
                    pair, is_k = ct % NPAIR, ct // NPAIR
                    dst = (kT if is_k else qT)[:, pair, ts0:ts0 + QTS]
                    nc.vector.tensor_scalar_add(dst, ps[:], bqk_sb[:, ct:ct + 1])

                # ---- v projection (token-major out) ----
                for sub in range(QTS // P):
                    ps = mm_ps.tile([P, VC], FP32, tag="mm512")
                    for kc in range(KC):
                        nc.tensor.matmul(
                            ps[:], xts[kc][:, sub * P:(sub + 1) * P],
                            wv_sb[:, kc, :], start=(kc == 0),
                            stop=(kc == KC - 1))
                    vt = tt * (QTS // P) + sub
                    nc.vector.tensor_add(
                        v_sb[:, vt, :, 0:DH],
                        ps[:].rearrange("p (h e) -> p h e", e=DH),
                        bv_rep[:].rearrange("p (h e) -> p h e", e=DH))

                # ---- attention for query tile tt, all head pairs ----
                j = tt
                nkt = 4 * (j + 1)  # causal: k tiles 0 .. nkt-1
                for pair in range(NPAIR):
                    at_A = at_ps.tile([P, QTS], FP32, tag="atA")
                    at_B = at_ps.tile([P, QTS], FP32, tag="atB")
                    for grp in range(nkt // 2):
                        sc = sc_ps.tile([P, 2048], FP32, tag="sc")
                        for i in range(2):
                            kt = 2 * grp + i
                            nc.tensor.matmul(
                                sc[:, i * 512:(i + 1) * 512],
                                kT[0:DH, pair, kt * P:(kt + 1) * P],
                                qT[0:DH, pair, ts0:ts0 + QTS],
                                start=True, stop=True)
                            nc.tensor.matmul(
                                sc[:, 1024 + i * 512:1024 + (i + 1) * 512],
                                kT[DH:P, pair, kt * P:(kt + 1) * P],
                                qT[DH:P, pair, ts0:ts0 + QTS],
                                start=True, stop=True)
                        pr = probs_pool.tile([P, 2048], BF16)
                        nc.scalar.activation(pr[:], sc[:], AF.Exp,
                                             scale=1.0 / np.sqrt(DH))
                        # causal fixups on diagonal tiles (k tile ki vs q tile)
                        for i in range(2):
                            kt = 2 * grp + i
                            dd = kt * P - ts0
                            if dd < 0:
                                continue  # strictly below diagonal: all valid
                            for h01 in range(2):
                                off = h01 * 1024 + i * 512
                                if dd > 0:
                                    nc.vector.memset(
                                        pr[:, off:off + dd], 0.0)
                                nc.vector.tensor_mul(
                                    pr[:, off + dd:off + dd + P],
                                    pr[:, off + dd:off + dd + P],
                                    band_sb[:])
                        for i in range(2):
                            kt = 2 * grp + i
                            for h01, at in ((0, at_A), (1, at_B)):
                                nc.tensor.matmul(
                                    at[0:DH + 1, :],
                                    v_sb[:, kt, 2 * pair + h01, :],
                                    pr[:, h01 * 1024 + i * 512:
                                       h01 * 1024 + (i + 1) * 512],
                                    start=(kt == 0), stop=(kt == nkt - 1))
                    # ---- normalize: a^T / rowsum, store to aTn ----
                    for h01, at in ((0, at_A), (1, at_B)):
                        sums = norm_pool.tile([1, QTS], FP32, tag="sums")
                        nc.vector.tensor_copy(sums[:], at[DH:DH + 1, :])
                        rs = norm_pool.tile([1, QTS], FP32, tag="rs")
                        nc.vector.reciprocal(rs[:], sums[:])
                        h = 2 * pair + h01
                        nc.sync.dma_start(out=rsd[h:h + 1, :], in_=rs[:])
                        rc = norm_pool.tile([P, QTS], FP32, tag="rc")
                        nc.sync.dma_start(
                            out=rc[:], in_=rsd[h:h + 1, :].to_broadcast((P, QTS)))
                        if h01 == 0:
                            nc.vector.tensor_mul(
                                aTn[0:DH, pair, ts0:ts0 + QTS],
                                at[0:DH, :], rc[0:DH, :])
                        else:
                            tmp = norm_pool.tile([DH, QTS], BF16, tag="tmpB")
                            nc.vector.tensor_mul(tmp[:], at[0:DH, :],
                                                 rc[0:DH, :])
                            nc.sync.dma_start(
                                out=aTn[DH:P, pair, ts0:ts0 + QTS],
                                in_=tmp[:])

                # ---- partial out-projection for this token tile ----
                for sub in range(QTS // P):
                    ysb = y_pool.tile([P, d], FP32)
                    t0 = ts0 + sub * P
                    for n in range(NOUT):
                        ps = mm_ps.tile([P, 512], FP32, tag="mm512")
                        for pc in range(PC):
                            nc.tensor.matmul(
                                ps[:], aTn[:, pc, t0:t0 + P],
                                wp_sb[:, pc, n * 512:(n + 1) * 512],
                                start=(pc == 0), stop=(pc == PC - 1))
                        nc.vector.tensor_add(ysb[:, n * 512:(n + 1) * 512],
                                             ps[:],
                                             bp_rep[:, n * 512:(n + 1) * 512])
                    nc.sync.dma_start(out=yp[t0:t0 + P, :], in_=ysb[:])

            if dbg:
                for name, src, dst in (("qT", qT, qTo), ("kT", kT, kTo),
                                       ("v", v_sb, vo), ("aTn", aTn, aTno)):
                    t = singles.tile(list(src.shape), FP32, tag="d" + name)
                    nc.vector.tensor_copy(t[:], src[:])
                    nc.sync.dma_start(out=dst[:], in_=t[:])

    nc.compile()
    return nc


_NC_CACHE = {}


def _get_nc():
    if "nc" not in _NC_CACHE:
        _NC_CACHE["nc"] = build_nc()
    return _NC_CACHE["nc"]


def make_in_maps(x, w_attn, b_attn, w_proj, b_proj):
    """Host-side sharding: batch c//2, head-group c%2."""
    VC = HPC * DH  # 512
    wq, wk, wv = w_attn[:, :D], w_attn[:, D:2 * D], w_attn[:, 2 * D:]
    bq, bk, bv = b_attn[:D], b_attn[D:2 * D], b_attn[2 * D:]
    in_maps = []
    for c in range(N_CORES):
        b, g = c // 2, c % 2
        sl = slice(g * VC, (g + 1) * VC)
        in_maps.append({
            "xb": np.ascontiguousarray(x[b]),
            "wqk": np.ascontiguousarray(
                np.concatenate([wq[:, sl], wk[:, sl]], axis=1)),
            "wv": np.ascontiguousarray(wv[:, sl]),
            "wp": np.ascontiguousarray(w_proj[g * VC:(g + 1) * VC, :]),
            "bqk": np.ascontiguousarray(
                np.concatenate([bq[sl], bk[sl]])),
            "bv": np.ascontiguousarray(bv[sl]),
            "bph": np.ascontiguousarray(b_proj * 0.5),
        })
    return in_maps


def kernel(x, w_attn, b_attn, w_proj, b_proj):
    x = np.asarray(x, dtype=np.float32)
    w_attn = np.asarray(w_attn, dtype=np.float32)
    b_attn = np.asarray(b_attn, dtype=np.float32)
    w_proj = np.asarray(w_proj, dtype=np.float32)
    b_proj = np.asarray(b_proj, dtype=np.float32)

    nc = _get_nc()
    in_maps = make_in_maps(x, w_attn, b_attn, w_proj, b_proj)
    res = run_bass_kernel_spmd(nc, in_maps, core_ids=list(range(N_CORES)))
    out = np.empty((B, S, D), dtype=np.float32)
    for b in range(B):
        out[b] = res.results[2 * b]["yp"] + res.results[2 * b + 1]["yp"]
    return out
